# revision 10
# baseline (speedup 1.0000x reference)
"""Trainium2 Bass kernel for nn_Block_51221779972150 (ViT-style transformer block).

Contract: kernel(**inputs) takes the FULL unsharded inputs from
reference.setup_inputs() and returns the FULL [4, 768, 36, 36] output.

Sharding (8 cores, dual-program token-split):
  Two Bass programs. Program A runs on cores 0-3 (one batch element each)
  and owns query chunks {0, 3} of the 4x324-token chunking; program B runs
  on cores 4-7 and owns chunks {1, 2}. Causal attention gives both 15
  k-tiles of work (3+12 vs 6+9) -> balanced. Each core computes patch
  embed + LN1 + K + V for every k-position its queries can see (B skips
  chunk 3 entirely), then Q/attention/out-proj/LN2/full-hidden MLP for its
  own chunks only. Outputs are disjoint column sets; the host concatenates.
  The two programs execute concurrently on disjoint core subsets.

On-chip layout: feature-major [E(partitions, 6 tiles of 128), T(free)].
All matmuls use float32r (fp32 stored, FP22 multiply, fp32 accumulate —
full PE rate for free-dim >= 256). Softmax has no max-subtraction
(scores ~ +-0.5, exp is safe); scores are computed transposed [k, q] so
exp output feeds attn@V directly with no transposes. Causal masks are
applied in-place on the exp output by GpSimd affine_select (keeps the
DVE free). Softmax denominators come from a ones-column in V;
per-column row broadcasts come from K=1 matmuls. tok / tok2 bounce
through DRAM scratch; MLP weights stream in 4 double-buffered groups.
"""

import sys

for _p in ("/opt/trn_rl_repo", "/root/.axon_site/_ro/trn_rl_repo"):
    if _p not in sys.path:
        sys.path.append(_p)

import numpy as np

import concourse.bass as bass
import concourse.mybir as mybir
import concourse.tile as tile
from concourse.bass_utils import run_bass_kernel_spmd

# ---------------- problem dims ----------------
P = 128
B = 4
C_IN = 3
IMG = 576
PATCH = 16
G = 36
E = 768
H = 12
HD = 64
T = G * G  # 1296
ET = E // P  # 6 e-tiles
CH = 324  # t-chunk (free-dim of most matmuls)
NCH = T // CH  # 4
KT = 108  # k-tile rows inside attention
NKT = T // KT  # 12
KPC = CH // KT  # k-tiles per chunk = 3
NP = H // 2  # 6 head pairs
MTF = 4 * E // P  # 24 m-tiles (full MLP hidden)
GRP = 6  # m-tiles per streamed MLP weight group
NG = MTF // GRP  # 4 groups
SCALE = float(E) ** -0.5
EPS = 1e-5
N_CORES = 8
OWN = {0: (0, 3), 1: (1, 2)}  # parity -> owned chunks

F32 = mybir.dt.float32
F32R = mybir.dt.float32r
AL = mybir.AluOpType
AF = mybir.ActivationFunctionType

PHASE_MARKS = []  # (phase_name, first_instruction_id) — filled during build


def build_program(parity: int = 0, reps: int = 1, split_waits: bool = True) -> bass.Bass:
    own = OWN[parity]
    # chunks whose tokens any of this core's queries can see (k coverage)
    n_kch = max(own) + 1  # A: 4, B: 3
    nc = bass.Bass()

    # ---- I/O ----
    xp_d = nc.declare_dram_parameter("xp", [E, T], F32R, isOutput=False)
    wc_d = nc.declare_dram_parameter("wc", [E, E], F32R, isOutput=False)
    wq_d = nc.declare_dram_parameter("wq", [E, E], F32R, isOutput=False)
    wk_d = nc.declare_dram_parameter("wk", [E, E], F32R, isOutput=False)
    wv_d = nc.declare_dram_parameter("wv", [E, E], F32R, isOutput=False)
    wo_d = nc.declare_dram_parameter("wo", [E, E], F32R, isOutput=False)
    w1_d = nc.declare_dram_parameter("w1f", [E, 4 * E], F32R, isOutput=False)
    w2_d = nc.declare_dram_parameter("w2f", [4 * E, E], F32R, isOutput=False)
    cb_d = nc.declare_dram_parameter("cb", [E], F32, isOutput=False)
    bo_d = nc.declare_dram_parameter("bo", [E], F32, isOutput=False)
    hb2_d = nc.declare_dram_parameter("hb2", [E], F32, isOutput=False)
    b1h_d = nc.declare_dram_parameter("b1f", [4 * E], F32, isOutput=False)
    g1_d = nc.declare_dram_parameter("g1", [E], F32, isOutput=False)
    bb1_d = nc.declare_dram_parameter("bb1", [E], F32, isOutput=False)
    g2_d = nc.declare_dram_parameter("g2", [E], F32, isOutput=False)
    bb2_d = nc.declare_dram_parameter("bb2", [E], F32, isOutput=False)
    cst2_d = nc.declare_dram_parameter("cst2", [P, 2], F32R, isOutput=False)
    cstr_d = nc.declare_dram_parameter("cstrow", [1, P], F32R, isOutput=False)
    out_d = nc.declare_dram_parameter("out", [E, T], F32, isOutput=True)

    # DRAM scratch
    tok_dr = nc.dram_tensor("tok_scratch", [P, ET, T], F32R)
    tok2_dr = nc.dram_tensor("tok2_scratch", [P, ET, NCH, CH], F32R)

    with nc.allow_low_precision(reason="fp32r matmul inputs"), tile.TileContext(nc) as tc:
        with (
            tc.tile_pool(name="big", bufs=1) as big,
            tc.tile_pool(name="wrk", bufs=2) as wrk,
            tc.tile_pool(name="cst", bufs=1) as cst,
            tc.tile_pool(name="mm", bufs=4, space="PSUM") as pmm,
            tc.tile_pool(name="acc", bufs=2, space="PSUM") as pacc,
            tc.tile_pool(name="den", bufs=2, space="PSUM") as pden,
        ):
            # ---------- constants ----------
            def load_cols(dram, o):
                t = cst.tile([P, o], F32, tag=f"c_{dram.name}")
                nc.gpsimd.dma_start(t[:], dram[:].rearrange("(o p) -> p o", p=P))
                return t

            cb_c = load_cols(cb_d, ET)
            bo_c = load_cols(bo_d, ET)
            hb2_c = load_cols(hb2_d, ET)
            g1_c = load_cols(g1_d, ET)
            bb1_c = load_cols(bb1_d, ET)
            g2_c = load_cols(g2_d, ET)
            bb2_c = load_cols(bb2_d, ET)
            b1f_c = load_cols(b1h_d, MTF)

            cst2 = cst.tile([P, 2], F32R, tag="cst2")
            nc.gpsimd.dma_start(cst2[:], cst2_d[:])
            ones_mean = cst2[:, 0:1]  # value 1/E
            ones_k = cst2[0:KT, 1:2]  # value 1.0
            ones_b = cst.tile([1, P], F32R, tag="ones_b")
            nc.gpsimd.dma_start(ones_b[:], cstr_d[:])
            eps_c = cst.tile([1, 1], F32, tag="eps")
            nc.gpsimd.memset(eps_c[:], EPS)

            for _rep in range(reps):
                PHASE_MARKS.append(("PH1_embed", nc.next_id()))
                # ---------- persistent tensors ----------
                h_t = big.tile([P, ET, T], F32R, tag="h")  # LN1 output
                kfm_t = big.tile([P, ET, T], F32R, tag="kw")  # K feature-major
                v_t = big.tile([P, NKT, H * (HD + 1)], F32R, tag="vw")  # V + ones col

                def load_w(dram, tag):
                    t = big.tile([P, ET, E], F32R, tag=tag)
                    nc.sync.dma_start(t[:], dram[:].rearrange("(o p) e -> p o e", p=P))
                    return t

                wc_t = load_w(wc_d, "wring")
                wo_t = load_w(wo_d, "wo")

                def ln_rows(s1_ps, s2_ps):
                    """psum rows S1 = mean(x), S2 = mean(x^2) ->
                    (rstd_bc, murstd_bc) psum [P, CH] broadcasts."""
                    mu_sb = wrk.tile([1, CH], F32, tag="murow")
                    nc.vector.tensor_copy(out=mu_sb[:], in_=s1_ps)
                    mu2 = wrk.tile([1, CH], F32, tag="row")
                    nc.vector.tensor_tensor(mu2[:], mu_sb[:], mu_sb[:], AL.mult)
                    var = wrk.tile([1, CH], F32, tag="row")
                    nc.vector.tensor_tensor(var[:], s2_ps, mu2[:], AL.subtract)
                    std = wrk.tile([1, CH], F32, tag="row")
                    nc.scalar.activation(std[:], var[:], AF.Sqrt, bias=eps_c[:])
                    rstd = wrk.tile([1, CH], F32R, tag="row")
                    nc.vector.reciprocal(rstd[:], std[:])
                    murstd = wrk.tile([1, CH], F32R, tag="row")
                    nc.vector.tensor_tensor(murstd[:], mu_sb[:], rstd[:], AL.mult)
                    rstd_bc = pden.tile([P, 512], F32, tag="den", name="rstd_bc")[:, :CH]
                    nc.tensor.matmul(rstd_bc[:], ones_b[:], rstd[:], start=True, stop=True)
                    mur_bc = pden.tile([P, 512], F32, tag="den", name="mur_bc")[:, :CH]
                    nc.tensor.matmul(mur_bc[:], ones_b[:], murstd[:], start=True, stop=True)
                    return rstd_bc, mur_bc

                # ================= PH1: patch embed + LN1 =================
                for ch in range(n_kch):
                    sc = bass.ds(ch * CH, CH)
                    xp_s = wrk.tile([P, ET, CH], F32R, tag="slab", bufs=2)
                    nc.sync.dma_start(
                        xp_s[:], xp_d[:].rearrange("(o p) t -> p o t", p=P)[:, :, sc]
                    )
                    tok_c = wrk.tile([P, ET, CH], F32R, tag="tc")
                    s1_ps = pden.tile([P, 512], F32, tag="den", name="s1_ps")[:, :CH]
                    s2_ps = pden.tile([P, 512], F32, tag="den", name="s2_ps")[:, :CH]
                    for eo in range(ET):
                        ps = pmm.tile([P, 512], F32, tag="mm", name="mmps")[:, :CH]
                        for ei in range(ET):
                            nc.tensor.matmul(
                                ps[:],
                                wc_t[:, ei, eo * P : (eo + 1) * P],
                                xp_s[:, ei, :],
                                start=(ei == 0),
                                stop=(ei == ET - 1),
                            )
                        nc.vector.tensor_scalar_add(
                            tok_c[:, eo, :], ps[:], cb_c[:, eo : eo + 1]
                        )
                        sq = wrk.tile([P, CH], F32R, tag="s1")
                        nc.scalar.activation(
                            sq[:], ps[:], AF.Square, bias=cb_c[:, eo : eo + 1]
                        )
                        nc.tensor.matmul(
                            s1_ps[0:1, :], ones_mean[:], tok_c[:, eo, :],
                            start=(eo == 0), stop=(eo == ET - 1),
                        )
                        nc.tensor.matmul(
                            s2_ps[0:1, :], ones_mean[:], sq[:],
                            start=(eo == 0), stop=(eo == ET - 1),
                        )
                    if ch in own:
                        nc.sync.dma_start(tok_dr[:, :, sc], tok_c[:])
                    rstd_bc, mur_bc = ln_rows(s1_ps[0:1, :], s2_ps[0:1, :])
                    for ei in range(ET):
                        hsl = h_t[:, ei, sc]
                        nc.vector.tensor_tensor(hsl, tok_c[:, ei, :], rstd_bc[:], AL.mult)
                        nc.vector.tensor_tensor(hsl, hsl, mur_bc[:], AL.subtract)
                        nc.vector.tensor_scalar(
                            hsl, hsl, g1_c[:, ei : ei + 1], bb1_c[:, ei : ei + 1],
                            AL.mult, AL.add,
                        )

                PHASE_MARKS.append(("PH2_KV", nc.next_id()))
                # ================= PH2: K and V projections =================
                wk_t = load_w(wk_d, "wring")
                for pp in range(ET):
                    for ch in range(n_kch):
                        sc = bass.ds(ch * CH, CH)
                        ps = pmm.tile([P, 512], F32, tag="mm", name="mmps")[:, :CH]
                        for ei in range(ET):
                            nc.tensor.matmul(
                                ps[:],
                                wk_t[:, ei, pp * P : (pp + 1) * P],
                                h_t[:, ei, sc],
                                start=(ei == 0),
                                stop=(ei == ET - 1),
                            )
                        nc.scalar.activation(kfm_t[:, pp, sc], ps[:], AF.Copy)

                wv_t = load_w(wv_d, "wring")
                EC = 384  # e_out chunk for V projection
                for tt in range(KPC * n_kch):
                    for ec in range(2):
                        ps = pmm.tile([P, 512], F32, tag="mm", name="mmps")[:, :EC]
                        for ei in range(ET):
                            nc.tensor.matmul(
                                ps[:KT, :],
                                h_t[:, ei, tt * KT : (tt + 1) * KT],
                                wv_t[:, ei, ec * EC : (ec + 1) * EC],
                                start=(ei == 0),
                                stop=(ei == ET - 1),
                            )
                        vrow = v_t[:KT, tt, :].rearrange("k (g c) -> k g c", c=HD + 1)
                        nc.scalar.activation(
                            vrow[:, 6 * ec : 6 * ec + 6, 0:HD],
                            ps[:KT, :].rearrange("k (g c) -> k g c", c=HD),
                            AF.Copy,
                        )
                    nc.vector.tensor_copy(
                        out=v_t[:KT, tt, HD :: HD + 1],
                        in_=ones_k[:KT, :].to_broadcast([KT, H]),
                    )

                PHASE_MARKS.append(("PH3_attn", nc.next_id()))
                # ================= PH3: attention + out-proj + residual =================
                wq_t = load_w(wq_d, "wring")
                for ci, ch in enumerate(own):
                    sc = bass.ds(ch * CH, CH)
                    nkt = KPC * (ch + 1)  # causal k-tiles for this chunk
                    ao_c = wrk.tile([P, ET, CH], F32R, tag="ao", bufs=1)
                    for pp in range(NP):
                        qps = pmm.tile([P, 512], F32, tag="mm", name="mmps")[:, :CH]
                        for ei in range(ET):
                            nc.tensor.matmul(
                                qps[:],
                                wq_t[:, ei, pp * P : (pp + 1) * P],
                                h_t[:, ei, sc],
                                start=(ei == 0),
                                stop=(ei == ET - 1),
                            )
                        q_pair = wrk.tile([P, CH], F32R, tag="qp")
                        nc.vector.tensor_copy(out=q_pair[:], in_=qps[:])

                        # f32r matmuls may only write PSUM at base partition 0,
                        # so each head accumulates at [0:64) and the odd head's
                        # normalized result is DMA-relocated to partitions 64-127.
                        ao_pss = [
                            pacc.tile([P, 512], F32, tag="acc", name="ao_ps")
                            for _ in range(2)
                        ]
                        # software pipeline: scores/exp for k-tile kt run while
                        # the AV accumulation consumes k-tile kt-1.
                        ets = {}

                        def do_av(kt, nkt=nkt, pp=pp, ao_pss=ao_pss, ets=ets):
                            for h2 in range(2):
                                head = 2 * pp + h2
                                nc.tensor.matmul(
                                    ao_pss[h2][0 : HD + 1, :CH],
                                    v_t[:KT, kt, head * (HD + 1) : (head + 1) * (HD + 1)],
                                    ets.pop((kt, h2))[:],
                                    start=(kt == 0),
                                    stop=(kt == nkt - 1),
                                )

                        for kt in range(nkt):
                            for h2 in range(2):
                                s_ps = pmm.tile([P, 512], F32, tag="mm", name="mmps")[:, :CH]
                                nc.tensor.matmul(
                                    s_ps[:KT, :],
                                    kfm_t[h2 * HD : (h2 + 1) * HD, pp, kt * KT : (kt + 1) * KT],
                                    q_pair[h2 * HD : (h2 + 1) * HD, :],
                                    start=True,
                                    stop=True,
                                    tile_position=(h2 * HD, 0),
                                )
                                et = wrk.tile([KT, CH], F32R, tag="expT", bufs=4)
                                nc.scalar.activation(et[:], s_ps[:KT, :], AF.Exp, scale=SCALE)
                                j = kt - KPC * ch
                                if j >= 0:  # diagonal tile: causal mask on GpSimd
                                    nc.gpsimd.affine_select(
                                        out=et[:],
                                        in_=et[:],
                                        compare_op=AL.is_ge,
                                        fill=0.0,
                                        base=-KT * j,
                                        pattern=[[1, CH]],
                                        channel_multiplier=-1,
                                    )
                                ets[(kt, h2)] = et
                            if kt >= 1:
                                do_av(kt - 1)
                        do_av(nkt - 1)
                        for h2 in range(2):
                            rc = wrk.tile([1, CH], F32R, tag="row")
                            nc.vector.reciprocal(rc[:], ao_pss[h2][HD : HD + 1, :CH])
                            rd_ps = pmm.tile([P, 512], F32, tag="mm", name="mmps")
                            nc.tensor.matmul(
                                rd_ps[0:HD, :CH], ones_b[:, 0:HD], rc[:],
                                start=True, stop=True,
                            )
                            rd_sb = wrk.tile([HD, CH], F32, tag="rdbc", bufs=2)
                            nc.vector.tensor_copy(out=rd_sb[:], in_=rd_ps[0:HD, :CH])
                            if h2 == 0:
                                nc.vector.tensor_tensor(
                                    ao_c[0:HD, pp, :], ao_pss[0][0:HD, :CH],
                                    rd_sb[:], AL.mult,
                                )
                            else:
                                hi = wrk.tile([HD, CH], F32R, tag="aohi", bufs=2)
                                nc.vector.tensor_tensor(
                                    hi[:], ao_pss[1][0:HD, :CH], rd_sb[:], AL.mult
                                )
                                nc.sync.dma_start(ao_c[HD:P, pp, :], hi[:])

                    # out-projection + residual: tok2 = tok + ao @ wo + bo
                    tokr = wrk.tile([P, ET, CH], F32R, tag="tc")
                    nc.sync.dma_start(tokr[:], tok_dr[:, :, sc])
                    for eo in range(ET):
                        ps = pmm.tile([P, 512], F32, tag="mm", name="mmps")[:, :CH]
                        for ei in range(ET):
                            nc.tensor.matmul(
                                ps[:],
                                wo_t[:, ei, eo * P : (eo + 1) * P],
                                ao_c[:, ei, :],
                                start=(ei == 0),
                                stop=(ei == ET - 1),
                            )
                        x = wrk.tile([P, CH], F32, tag="s1")
                        nc.vector.tensor_scalar_add(x[:], ps[:], bo_c[:, eo : eo + 1])
                        nc.vector.tensor_tensor(
                            tokr[:, eo, :], tokr[:, eo, :], x[:], AL.add
                        )
                    nc.sync.dma_start(tok2_dr[:, :, ch, :], tokr[:])

                PHASE_MARKS.append(("PH4_mlp", nc.next_id()))
                # ================= PH4: LN2 + full-hidden MLP + output =================
                h2_t = big.tile([P, 2, ET, CH], F32R, tag="wring")
                for ci, ch in enumerate(own):
                    t2r = wrk.tile([P, ET, CH], F32R, tag="tc")
                    nc.sync.dma_start(t2r[:], tok2_dr[:, :, ch, :])
                    s1_ps = pden.tile([P, 512], F32, tag="den", name="s1_ps")[:, :CH]
                    s2_ps = pden.tile([P, 512], F32, tag="den", name="s2_ps")[:, :CH]
                    for ei in range(ET):
                        sq = wrk.tile([P, CH], F32R, tag="s1")
                        nc.scalar.activation(sq[:], t2r[:, ei, :], AF.Square)
                        nc.tensor.matmul(
                            s1_ps[0:1, :], ones_mean[:], t2r[:, ei, :],
                            start=(ei == 0), stop=(ei == ET - 1),
                        )
                        nc.tensor.matmul(
                            s2_ps[0:1, :], ones_mean[:], sq[:],
                            start=(ei == 0), stop=(ei == ET - 1),
                        )
                    rstd_bc, mur_bc = ln_rows(s1_ps[0:1, :], s2_ps[0:1, :])
                    for ei in range(ET):
                        hsl = h2_t[:, ci, ei, :]
                        nc.vector.tensor_tensor(hsl, t2r[:, ei, :], rstd_bc[:], AL.mult)
                        nc.vector.tensor_tensor(hsl, hsl, mur_bc[:], AL.subtract)
                        nc.vector.tensor_scalar(
                            hsl, hsl, g2_c[:, ei : ei + 1], bb2_c[:, ei : ei + 1],
                            AL.mult, AL.add,
                        )

                # fc2 partial sums accumulate in SBUF across weight groups
                acc_t = big.tile([P, 2, ET, CH], F32, tag="wo")
                for g in range(NG):
                    gm = bass.ds(g * GRP * P, GRP * P)
                    w1g = big.tile([P, ET, GRP * P], F32R, tag="kw", bufs=1)
                    nc.sync.dma_start(
                        w1g[:], w1_d[:].rearrange("(o p) m -> p o m", p=P)[:, :, gm]
                    )
                    w2g = big.tile([P, GRP, E], F32R, tag="vw", bufs=1)
                    nc.sync.dma_start(
                        w2g[:],
                        w2_d[gm, :].rearrange("(o p) e -> p o e", p=P),
                    )
                    for ci, ch in enumerate(own):
                        a_c = wrk.tile([P, GRP, CH], F32R, tag="tc")
                        if g == NG - 1:
                            t2o = wrk.tile([P, ET, CH], F32R, tag="tc")
                            nc.sync.dma_start(t2o[:], tok2_dr[:, :, ch, :])
                        for m in range(GRP):
                            ps = pmm.tile([P, 512], F32, tag="mm", name="mmps")[:, :CH]
                            for ei in range(ET):
                                nc.tensor.matmul(
                                    ps[:],
                                    w1g[:, ei, m * P : (m + 1) * P],
                                    h2_t[:, ci, ei, :],
                                    start=(ei == 0),
                                    stop=(ei == ET - 1),
                                )
                            nc.scalar.activation(
                                a_c[:, m, :], ps[:], AF.Relu,
                                bias=b1f_c[:, g * GRP + m : g * GRP + m + 1],
                            )
                        for eo in range(ET):
                            ps = pmm.tile([P, 512], F32, tag="mm", name="mmps")[:, :CH]
                            for m in range(GRP):
                                nc.tensor.matmul(
                                    ps[:],
                                    w2g[:, m, eo * P : (eo + 1) * P],
                                    a_c[:, m, :],
                                    start=(m == 0),
                                    stop=(m == GRP - 1),
                                )
                            if g == 0:
                                nc.vector.tensor_scalar_add(
                                    acc_t[:, ci, eo, :], ps[:], hb2_c[:, eo : eo + 1]
                                )
                            elif g < NG - 1:
                                nc.vector.tensor_tensor(
                                    acc_t[:, ci, eo, :], acc_t[:, ci, eo, :], ps[:],
                                    AL.add,
                                )
                            else:
                                ob = wrk.tile([P, CH], F32, tag="s1")
                                nc.vector.tensor_tensor(
                                    ob[:], acc_t[:, ci, eo, :], ps[:], AL.add
                                )
                                nc.vector.tensor_tensor(
                                    ob[:], ob[:], t2o[:, eo, :], AL.add
                                )
                                nc.sync.dma_start(
                                    out_d[eo * P : (eo + 1) * P, ch * CH : (ch + 1) * CH],
                                    ob[:],
                                )

    if split_waits:
        _split_matmul_waits(nc)
    return nc


def _split_matmul_waits(nc: bass.Bass) -> None:
    """The fused f32r Matmult (S3_LW) carries at most ONE sync wait in
    walrus codegen. Peel extra waits onto PE NoOps inserted just before
    the matmul (same engine -> sequencer order preserves semantics)."""
    f = nc.m.functions[0]
    nid = 0
    for blk in f.blocks:
        insts = list(blk.instructions)
        out = []
        changed = False
        for inst in insts:
            if (
                inst.sync_info is not None
                and len(inst.sync_info.on_wait) > 1
                and type(inst).__name__ != "InstCall"
            ):
                waits = list(inst.sync_info.on_wait)
                for w in waits[:-1]:
                    nop = mybir.InstNoOp(name=f"I-mmwait{nid}", engine=inst.engine)
                    nid += 1
                    nop.sync_info = mybir.SyncInfo(on_wait=[w], on_update=[])
                    out.append(nop)
                inst.sync_info = mybir.SyncInfo(
                    on_wait=[waits[-1]], on_update=list(inst.sync_info.on_update)
                )
                changed = True
            out.append(inst)
        if changed:
            blk.instructions = out


_PROGRAMS = {}


def _get_program(parity: int) -> bass.Bass:
    if parity not in _PROGRAMS:
        _PROGRAMS[parity] = build_program(parity)
    return _PROGRAMS[parity]


def host_inputs(inputs: dict, core: int) -> dict:
    """Per-core in_map. Cores 0-3: program A (parity 0), batch = core.
    Cores 4-7: program B (parity 1), batch = core - 4."""
    b = core % 4
    f = lambda a: np.ascontiguousarray(np.asarray(a), dtype=np.float32)
    x = f(inputs["x"])  # [4, 3, 576, 576]
    xp = np.ascontiguousarray(
        x[b].reshape(C_IN, G, PATCH, G, PATCH).transpose(0, 2, 4, 1, 3).reshape(E, T)
    )
    wc = np.ascontiguousarray(f(inputs["conv_w"]).reshape(E, E).T)  # [(cpq), e]
    return {
        "xp": xp,
        "wc": wc,
        "wq": f(inputs["wq"]),
        "wk": f(inputs["wk"]),
        "wv": f(inputs["wv"]),
        "wo": f(inputs["wo"]),
        "w1f": f(inputs["w1"]),
        "w2f": f(inputs["w2"]),
        "cb": f(inputs["conv_b"]),
        "bo": f(inputs["bo"]),
        "hb2": f(inputs["b2"]),
        "b1f": f(inputs["b1"]),
        "g1": f(inputs["ln1_g"]),
        "bb1": f(inputs["ln1_b"]),
        "g2": f(inputs["ln2_g"]),
        "bb2": f(inputs["ln2_b"]),
        "cst2": _cst2(),
        "cstrow": np.ones((1, P), np.float32),
    }


def _cst2() -> np.ndarray:
    c = np.ones((P, 2), np.float32)
    c[:, 0] = 1.0 / E
    return c


def combine_outputs(res_a: list, res_b: list) -> np.ndarray:
    """Gather each batch's owned column sets from the A and B cores."""
    out = np.empty((B, E, T), np.float32)
    for b in range(B):
        oa = res_a[b]["out"].astype(np.float32)
        ob = res_b[b]["out"].astype(np.float32)
        for ch in OWN[0]:
            out[b, :, ch * CH : (ch + 1) * CH] = oa[:, ch * CH : (ch + 1) * CH]
        for ch in OWN[1]:
            out[b, :, ch * CH : (ch + 1) * CH] = ob[:, ch * CH : (ch + 1) * CH]
    return out.reshape(B, E, G, G)


def kernel(**inputs) -> np.ndarray:
    in_maps = [host_inputs(inputs, c) for c in range(N_CORES)]
    res_a = run_bass_kernel_spmd(_get_program(0), in_maps[:4], [0, 1, 2, 3])
    res_b = run_bass_kernel_spmd(_get_program(1), in_maps[4:], [0, 1, 2, 3])
    return combine_outputs(res_a.results, res_b.results)


if __name__ == "__main__":
    nc = build_program(0)
    print("program A built ok")
    nc = build_program(1)
    print("program B built ok")
